# revision 5
# baseline (speedup 1.0000x reference)
"""Trainium2 Bass kernel for nn_DenseAttentionMultiHead (dense_transformer).

Reference computation (B=4, S=2048, H=2048, 16 heads, D=128, no softmax):
    x   = rope(hidden_states)                       # [B, S, H]
    q   = x @ W.T
    out = ((q_h @ k_h^T) @ k_h  per head)           # k == roped x heads

Key algebraic rewrite (valid because there is no softmax):
    (Q K^T) K == Q (K^T K)
so the [S, S] intermediate collapses to a [128, 128] Gram matrix per head,
cutting FLOPs ~2.7x. Per-core work is then dominated by the q-projection.

Sharding (8 cores): core c -> (batch b = c // 2, head-group g = c % 2).
Each core computes q = rope(X_b) @ W_g^T for its 1024 output columns and the
attention for its 8 heads; outputs are disjoint -> no collectives.

SPMD trick: the program always treats rows [0:1024] of its (transposed) input
as "its" heads.  The host rolls the H axis of hidden^T and the K axis of W_g^T
by 1024*g so one program works for both head-groups (contraction order is
permutation-invariant).

Schedule design (from perfetto traces):
  - input DMA (25MB fp32) paces the first ~half of the kernel; rope (DVE),
    converts (ACT) and a k-outer partial q-projection (PE, 7 PSUM banks)
    hide underneath it.
  - K^T chunk transposes go through the DMA XBAR (dma_start_transpose),
    not the PE, freeing PSUM banks and psum->sbuf copies.
  - loads are split across the sync and gpsimd DMA queues; swap copies are
    issued from gpsimd so the load queues never block on compute deps.
"""

from contextlib import ExitStack

import numpy as np

import concourse.bass as bass
import concourse.tile as tile
from concourse import bacc, mybir
from concourse.bass import ts
from concourse.bass_utils import run_bass_kernel_spmd

B, S, H = 4, 2048, 2048
NH, D = 16, 128
HPC = 8  # heads per core
GCOLS = HPC * D  # 1024 q-columns per core
NKT = H // 128  # 16 partition tiles along H
NMT = GCOLS // 128  # 8 q-column tiles
SC = 512  # matmul moving free-dim (one PSUM bank)
NSC = S // SC  # 4
NMA = 7  # m-tiles accumulated k-outer during the load phase
F32 = mybir.dt.float32
BF16 = mybir.dt.bfloat16


def build_kernel(ctx: ExitStack, tc: tile.TileContext, xt, wt, cosT, sinT, outT):
    nc = tc.nc

    p_xst = ctx.enter_context(tc.tile_pool(name="xst", bufs=2))
    p_wst = ctx.enter_context(tc.tile_pool(name="wst", bufs=1))
    p_rope = ctx.enter_context(tc.tile_pool(name="rope", bufs=2))
    p_xtr = ctx.enter_context(tc.tile_pool(name="xtr", bufs=NKT))
    p_w = ctx.enter_context(tc.tile_pool(name="wpool", bufs=NKT))
    p_qt = ctx.enter_context(tc.tile_pool(name="qtpool", bufs=NMT))
    p_cs = ctx.enter_context(tc.tile_pool(name="cs", bufs=1))
    p_k = ctx.enter_context(tc.tile_pool(name="kpool", bufs=2))
    p_m = ctx.enter_context(tc.tile_pool(name="mpool", bufs=2))
    p_ost = ctx.enter_context(tc.tile_pool(name="ost", bufs=2))
    ps_mm = ctx.enter_context(tc.tile_pool(name="psmm", bufs=7, space="PSUM"))
    ps_m = ctx.enter_context(tc.tile_pool(name="psm", bufs=1, space="PSUM"))

    # --- rope coefficient tiles: CC = [cos; cos], SS = [-sin; sin], bf16 [128, S]
    cs_f = p_xst.tile([128, S], F32, tag="stage")
    nc.sync.dma_start(out=cs_f[0:64, :], in_=cosT[:, :])
    nc.sync.dma_start(out=cs_f[64:128, :], in_=cosT[:, :])
    cc = p_cs.tile([128, S], BF16, tag="cc")
    nc.scalar.copy(cc[:], cs_f[:])

    ss_f = p_xst.tile([128, S], F32, tag="stage")
    nc.sync.dma_start(out=ss_f[0:64, :], in_=sinT[:, :])
    nc.sync.dma_start(out=ss_f[64:128, :], in_=sinT[:, :])
    ssg = p_cs.tile([128, S], BF16, tag="ss")
    nc.scalar.mul(ssg[0:64, :], ss_f[0:64, :], -1.0)
    nc.scalar.copy(ssg[64:128, :], ss_f[64:128, :])

    # --- streamed load + rope + partial q-projection (k-outer over 7 m-tiles)
    # rope per 128-row head tile: out = x * CC + swap_halves(x) * SS   (DVE x3)
    xtr = []
    wb = []
    qtA = [
        ps_mm.tile([128, SC], F32, tag="mm", name=f"psA{m}") for m in range(NMA)
    ]
    for kt in range(NKT):
        xf = p_xst.tile([128, S], F32, tag="stage", name=f"xf{kt}")
        eng = nc.sync if kt % 2 == 0 else nc.gpsimd
        eng.dma_start(out=xf[:], in_=xt[ts(kt, 128), :])
        wf = p_wst.tile([128, GCOLS], F32, tag="wstage", name=f"wf{kt}")
        nc.sync.dma_start(out=wf[:], in_=wt[ts(kt, 128), :])

        xb = p_rope.tile([128, S], BF16, tag="xb", name=f"xb{kt}")
        nc.scalar.copy(xb[:], xf[:])
        sw = p_rope.tile([128, S], BF16, tag="sw", name=f"sw{kt}")
        nc.gpsimd.dma_start(out=sw[0:64, :], in_=xb[64:128, :])
        nc.gpsimd.dma_start(out=sw[64:128, :], in_=xb[0:64, :])
        pterm = p_rope.tile([128, S], BF16, tag="pterm", name=f"pt{kt}", bufs=1)
        nc.vector.tensor_mul(pterm[:], xb[:], cc[:])
        nc.vector.tensor_mul(xb[:], sw[:], ssg[:])  # second product reuses xb
        xr = p_xtr.tile([128, S], BF16, tag="xtr", name=f"xtr{kt}")
        nc.vector.tensor_add(xr[:], pterm[:], xb[:])
        xtr.append(xr)

        wtile = p_w.tile([128, GCOLS], BF16, tag="wb", name=f"wb{kt}")
        nc.scalar.copy(wtile[:], wf[:])
        wb.append(wtile)

        # partial q^T accumulation for m-tiles 0..NMA-1, s-chunk 0
        for m in range(NMA):
            nc.tensor.matmul(
                qtA[m][:],
                wtile[:, ts(m, 128)],
                xr[:, ts(0, SC)],
                start=(kt == 0),
                stop=(kt == NKT - 1),
            )

    qt = [
        p_qt.tile([128, S], BF16, tag="qt", name=f"qt{m}") for m in range(NMT)
    ]
    for m in range(NMA):
        nc.scalar.copy(qt[m][:, ts(0, SC)], qtA[m][:])

    # --- per head: K^T chunks via DMA-XBAR transpose, Gram matrix M = K^T K
    msb = []
    for hl in range(HPC):
        ktn = p_k.tile([128, NKT, 128], BF16, tag="ktn", name=f"ktn{hl}")
        nc.sync.dma_start_transpose(ktn[:], xtr[hl][:])
        mps = ps_m.tile([128, 128], F32, tag="mg", name=f"mps{hl}")
        for cq in range(NKT):
            nc.tensor.matmul(
                mps[:],
                ktn[:, cq, :],
                ktn[:, cq, :],
                start=(cq == 0),
                stop=(cq == NKT - 1),
            )
        ms = p_m.tile([128, 128], BF16, tag="msb", name=f"msb{hl}")
        nc.scalar.copy(ms[:], mps[:])
        msb.append(ms)

    # --- remaining q^T projection (k-inner)
    rest = [(0, m) for m in range(NMA, NMT)] + [
        (sc, m) for sc in range(1, NSC) for m in range(NMT)
    ]
    for sc, m in rest:
        ps = ps_mm.tile([128, SC], F32, tag="mm", name=f"psq{sc}_{m}")
        for kt in range(NKT):
            nc.tensor.matmul(
                ps[:],
                wb[kt][:, ts(m, 128)],
                xtr[kt][:, ts(sc, SC)],
                start=(kt == 0),
                stop=(kt == NKT - 1),
            )
        if m % 2 == 0:
            nc.scalar.copy(qt[m][:, ts(sc, SC)], ps[:])
        else:
            nc.vector.tensor_copy(qt[m][:, ts(sc, SC)], ps[:])

    # --- out^T = M^T @ q^T per head, streamed straight out to DRAM
    for hl in range(HPC):
        for sc in range(NSC):
            ops = ps_mm.tile([128, SC], F32, tag="mm", name=f"pso{hl}_{sc}")
            nc.tensor.matmul(
                ops[:], msb[hl][:], qt[hl][:, ts(sc, SC)], start=True, stop=True
            )
            ot = p_ost.tile([128, SC], F32, tag="ost", name=f"ot{hl}_{sc}")
            if sc % 2 == 0:
                nc.scalar.copy(ot[:], ops[:])
            else:
                nc.vector.tensor_copy(ot[:], ops[:])
            nc.sync.dma_start(out=outT[ts(hl, 128), ts(sc, SC)], in_=ot[:])


_NC_CACHE = {}


def build_nc():
    if "nc" in _NC_CACHE:
        return _NC_CACHE["nc"]
    nc = bacc.Bacc("TRN2", target_bir_lowering=False, debug=False)
    xt = nc.dram_tensor("xt", [H, S], F32, kind="ExternalInput").ap()
    wt = nc.dram_tensor("wt", [H, GCOLS], F32, kind="ExternalInput").ap()
    cosT = nc.dram_tensor("cost", [64, S], F32, kind="ExternalInput").ap()
    sinT = nc.dram_tensor("sint", [64, S], F32, kind="ExternalInput").ap()
    outT = nc.dram_tensor("outT", [GCOLS, S], F32, kind="ExternalOutput").ap()
    with tile.TileContext(nc) as tc:
        with ExitStack() as ctx:
            build_kernel(ctx, tc, xt, wt, cosT, sinT, outT)
    nc.compile()
    _NC_CACHE["nc"] = nc
    return nc


def make_in_maps(hidden_states, W, cos, sin):
    hidden_states = np.asarray(hidden_states, dtype=np.float32)
    W = np.asarray(W, dtype=np.float32)
    cosT = np.ascontiguousarray(np.asarray(cos, dtype=np.float32).T)
    sinT = np.ascontiguousarray(np.asarray(sin, dtype=np.float32).T)
    in_maps = []
    for c in range(8):
        b, g = c // 2, c % 2
        roll = GCOLS * g
        xt = hidden_states[b].T  # [H, S]
        wt = W[GCOLS * g : GCOLS * (g + 1), :].T  # [H, 1024]
        if roll:
            xt = np.roll(xt, -roll, axis=0)
            wt = np.roll(wt, -roll, axis=0)
        in_maps.append(
            {
                "xt": np.ascontiguousarray(xt),
                "wt": np.ascontiguousarray(wt),
                "cost": cosT,
                "sint": sinT,
            }
        )
    return in_maps


def run(hidden_states, W, cos, sin, trace=False):
    nc = build_nc()
    in_maps = make_in_maps(hidden_states, W, cos, sin)
    res = run_bass_kernel_spmd(nc, in_maps, list(range(8)), trace=trace)
    out = np.empty((B, S, H), np.float32)
    for c in range(8):
        b, g = c // 2, c % 2
        out[b][:, GCOLS * g : GCOLS * (g + 1)] = res.results[c]["outT"].T
    return out, res


def kernel(hidden_states, W, cos, sin):
    out, _ = run(hidden_states, W, cos, sin, trace=False)
    return out
